# revision 24
# baseline (speedup 1.0000x reference)
"""AGF sparse attention (top-k=64 mask + softmax + 3-term polynomial filter)
on 8 TRN2 NeuronCores.

Sharding: core c -> batch b = c//2, head-group hg = c%2 (4 of 8 heads).
Each core runs the full per-(b, head-group) pipeline on-device:
  qkv projection -> scores (fp32r matmuls) -> exact-ish top-64 threshold via
  chunked max8 + match_replace -> masked softmax (unnormalized, fused
  mask*exp + row-sum) -> A^T via PE transposes -> 3x (A @ z) recurrence with
  per-token 1/rowsum normalization folded into the transpose-back copies ->
  partial output projection (+ 0.5*bout per core).
Host only shards inputs and sums the two partial outputs per batch element.
"""

import sys

sys.path.insert(0, "/opt/trn_rl_repo")

from contextlib import ExitStack  # noqa: E402

import numpy as np  # noqa: E402
import ml_dtypes  # noqa: E402

import concourse.bass as bass  # noqa: E402
import concourse.tile as tile  # noqa: E402
from concourse import bacc, mybir  # noqa: E402
from concourse.bass_utils import run_bass_kernel_spmd  # noqa: E402

FP = mybir.dt.float32
FPR = mybir.dt.float32r
BF = mybir.dt.bfloat16
AF = mybir.ActivationFunctionType
ALU = mybir.AluOpType

N, DIM = 2048, 512
H, HL, DH = 8, 4, 64  # total heads, local heads per core, head dim
NT = N // 128  # 16 token tiles
CC = DIM // 128  # 4 contraction chunks
ORDER = 3
NEG = -1.0e30
SCALE = DH**-0.5  # 0.125


def _build():
    nc = bacc.Bacc(
        "TRN2", target_bir_lowering=False, debug=False, num_devices=8
    )
    xt_d = nc.dram_tensor("xt", [DIM, N], FP, kind="ExternalInput")
    wqk_d = nc.dram_tensor("wqk", [DIM, 512], FP, kind="ExternalInput")
    wv_d = nc.dram_tensor("wv", [DIM, 256], FP, kind="ExternalInput")
    bqk_d = nc.dram_tensor("bqk", [512], FP, kind="ExternalInput")
    bv_d = nc.dram_tensor("bv", [1, 256], FP, kind="ExternalInput")
    wout_d = nc.dram_tensor("wout", [256, DIM], FP, kind="ExternalInput")
    bout_d = nc.dram_tensor("bout", [1, DIM], FP, kind="ExternalInput")
    ar_d = nc.dram_tensor("araw", [1, ORDER * HL], FP, kind="ExternalInput")
    out_d = nc.dram_tensor("out", [N, DIM], FP, kind="ExternalOutput")

    ident_bf_d = nc.inline_tensor(
        np.eye(128, dtype=ml_dtypes.bfloat16), name="identbf"
    )
    ident_f_d = nc.inline_tensor(np.eye(128, dtype=np.float32), name="identf")
    ones_d = nc.inline_tensor(np.ones((1, 128), np.float32), name="ones1")
    half_d = nc.inline_tensor(
        np.full((1, 128), 0.5, np.float32), name="half1"
    )

    with tile.TileContext(nc) as tc, ExitStack() as ctx:
        consts = ctx.enter_context(tc.tile_pool(name="consts", bufs=1))
        pw = ctx.enter_context(tc.tile_pool(name="weights", bufs=1))
        pqk = ctx.enter_context(tc.tile_pool(name="qkT", bufs=1))
        pv = ctx.enter_context(tc.tile_pool(name="vsb", bufs=1))
        pres = ctx.enter_context(tc.tile_pool(name="res", bufs=1))
        psum = ctx.enter_context(tc.tile_pool(name="psum", bufs=2, space="PSUM"))
        pS = ctx.enter_context(tc.tile_pool(name="pS", bufs=2))
        pA = ctx.enter_context(tc.tile_pool(name="pA", bufs=4))
        pzz = ctx.enter_context(tc.tile_pool(name="pzz", bufs=2))

        ident_bf = consts.tile([128, 128], BF)
        nc.sync.dma_start(ident_bf[:], ident_bf_d.ap())
        ident_f = consts.tile([128, 128], FP)
        nc.sync.dma_start(ident_f[:], ident_f_d.ap())
        ones_r = consts.tile([1, 128], FP)
        nc.sync.dma_start(ones_r[:], ones_d.ap())
        half_r = consts.tile([1, 128], FP)
        nc.sync.dma_start(half_r[:], half_d.ap())
        bvrow = consts.tile([1, 256], FP)
        nc.sync.dma_start(bvrow[:], bv_d.ap())
        boutrow = consts.tile([1, 512], FP)
        nc.sync.dma_start(boutrow[:], bout_d.ap())
        bqk_sb = consts.tile([128, 4], FP)
        nc.sync.dma_start(
            bqk_sb[:], bqk_d.ap().rearrange("(f p) -> p f", p=128)
        )
        araw_t = consts.tile([1, ORDER * HL], FP)
        nc.sync.dma_start(araw_t[:], ar_d.ap())
        alpha_g = consts.tile([1, ORDER * HL], FP)
        nc.scalar.activation(alpha_g[:], araw_t[:], AF.Gelu)
        alpha_sb = consts.tile([128, ORDER * HL], FP)
        nc.gpsimd.partition_broadcast(alpha_sb[:], alpha_g[:])

        wqk_sb = []
        wv_sb = []
        for c in range(CC):
            t = pw.tile([128, 512], FP, tag=f"wqk{c}", name=f"wqk{c}")
            nc.sync.dma_start(t[:], wqk_d.ap()[c * 128 : (c + 1) * 128, :])
            wqk_sb.append(t)
            t = pw.tile([128, 256], FP, tag=f"wv{c}", name=f"wv{c}")
            nc.sync.dma_start(t[:], wv_d.ap()[c * 128 : (c + 1) * 128, :])
            wv_sb.append(t)
        wout_sb = []
        for f in range(2):
            t = pw.tile([128, 512], FP, tag=f"wo{f}", name=f"wo{f}")
            nc.sync.dma_start(t[:], wout_d.ap()[f * 128 : (f + 1) * 128, :])
            wout_sb.append(t)

        # qkT tiles: ft 0..1 = q^T (heads 0-1, 2-3), ft 2..3 = k^T
        qkT = [pqk.tile([128, N], FP, tag=f"qkT{i}", name=f"qkT{i}") for i in range(4)]
        v_sb = pv.tile([128, NT, 256], BF)  # v rows, packed [t_lo, t_hi, f]
        res_row = pres.tile([128, NT, 256], FP)  # sum_r alpha_r z_r (rows)

        # ---- phase 1: load x, build x^T; phase 2: projections
        with tc.tile_pool(name="xload", bufs=1) as px:
            xT = [px.tile([128, N], FP, tag=f"xT{c}", name=f"xT{c}") for c in range(CC)]
            for c in range(CC):
                nc.sync.dma_start(
                    xT[c][:], xt_d.ap()[c * 128 : (c + 1) * 128, :]
                )

            for ft in (0, 2, 1, 3):
                for tq in range(4):
                    ps = psum.tile([128, 512], FP, tag="qk", bufs=1, name="psqk")
                    for c in range(CC):
                        nc.tensor.matmul(
                            ps[:],
                            wqk_sb[c][:, ft * 128 : (ft + 1) * 128],
                            xT[c][:, tq * 512 : (tq + 1) * 512],
                            start=(c == 0),
                            stop=(c == CC - 1),
                        )
                    nc.scalar.activation(
                        qkT[ft][:, tq * 512 : (tq + 1) * 512],
                        ps[:],
                        AF.Identity,
                        bias=bqk_sb[:, ft : ft + 1],
                    )
            for it in range(NT):
                psv = psum.tile([128, 512], FP, tag="gen", name="psg")
                ps = psv[:, 0:256]
                nc.tensor.matmul(
                    ps,
                    ones_r[:],
                    bvrow[:],
                    start=True,
                    stop=False,
                )
                for c in range(CC):
                    nc.tensor.matmul(
                        ps,
                        xT[c][:, it * 128 : (it + 1) * 128],
                        wv_sb[c][:],
                        start=False,
                        stop=(c == CC - 1),
                    )
                nc.scalar.copy(v_sb[:, it, :], ps)

        # ---- head loop
        with tc.tile_pool(name="pAT", bufs=1) as pAT:
            for hl in range(HL):
                qt_tile = qkT[hl // 2]
                kt_tile = qkT[2 + hl // 2]
                po = (hl % 2) * 64
                AT = pAT.tile([128, NT, N], BF, tag="AT")
                rd_all = pzz.tile([128, NT], FP, tag="rd")
                import contextlib
                for it in range(NT):
                    S_sb = pS.tile([128, N], FP, tag="S")
                    hoist = (
                        tc.high_priority() if hl == 0
                        else contextlib.nullcontext()
                    )
                    with hoist:
                        for jc in range(4):
                            ps = psum.tile([128, 512], FP, tag="S", name="psS")
                            nc.tensor.matmul(
                                ps[:],
                                qt_tile[
                                    po : po + 64, it * 128 : (it + 1) * 128
                                ],
                                kt_tile[
                                    po : po + 64, jc * 512 : (jc + 1) * 512
                                ],
                                start=True,
                                stop=True,
                            )
                            nc.scalar.copy(
                                S_sb[:, jc * 512 : (jc + 1) * 512], ps[:]
                            )
                    # top-64 threshold: chunk=64 top-8 candidates, then
                    # 8x (max8 + match_replace) on the 256 candidates
                    C = pS.tile([128, 256], FP, tag="C")
                    for ch in range(32):
                        nc.vector.max(
                            C[:, ch * 8 : (ch + 1) * 8],
                            S_sb[:, ch * 64 : (ch + 1) * 64],
                        )
                    mneg = pS.tile([128, 1], FP, tag="mn")
                    mrow = pS.tile([128, 1], FP, tag="mr")
                    C3 = C[:].rearrange("p (c e) -> p c e", e=8)
                    nc.vector.tensor_reduce(
                        mrow[:], C3[:, :, 0], axis=mybir.AxisListType.X,
                        op=ALU.max,
                    )
                    nc.vector.tensor_scalar_mul(mneg[:], mrow[:], -SCALE)
                    m8 = pS.tile([128, 8], FP, tag="m8")
                    for itr in range(8):
                        nc.vector.max(m8[:], C[:])
                        if itr < 7:
                            nc.vector.match_replace(C[:], m8[:], C[:], NEG)
                    tau = pS.tile([128, 1], FP, tag="tau")
                    nc.scalar.activation(
                        tau[:], m8[:, 7:8], AF.Exp,
                        bias=mneg[:, 0:1], scale=SCALE,
                    )
                    E_sb = pS.tile([128, N], FP, tag="E")
                    nc.scalar.activation(
                        E_sb[:], S_sb[:], AF.Exp,
                        bias=mneg[:, 0:1], scale=SCALE,
                    )
                    A_bf = pA.tile([128, N], BF, tag="A")
                    dsum = pS.tile([128, 1], FP, tag="d")
                    nc.vector.scalar_tensor_tensor(
                        A_bf[:],
                        E_sb[:],
                        tau[:, 0:1],
                        E_sb[:],
                        op0=ALU.is_ge,
                        op1=ALU.mult,
                        accum_out=dsum[:],
                    )
                    nc.vector.reciprocal(rd_all[:, it : it + 1], dsum[:])
                    for g in range(4):
                        nc.sync.dma_start_transpose(
                            AT[:, g * 4 : (g + 1) * 4,
                               it * 128 : (it + 1) * 128],
                            A_bf[:, g * 512 : (g + 1) * 512],
                        )
                # recurrence: z_r = rd * (A_u @ z_{r-1}), res += alpha_r z_r
                zprev = None
                for r in range(ORDER):
                    ztmp = pzz.tile([64, N], BF, tag="ztmp")
                    for nck in range(4):
                        psz = psum.tile([64, 512], FP, tag="z", name="psz")
                        for jt in range(NT):
                            lhsT = (
                                v_sb[:, jt, hl * 64 : (hl + 1) * 64]
                                if r == 0
                                else zprev[:, jt, :]
                            )
                            nc.tensor.matmul(
                                psz[:],
                                lhsT,
                                AT[:, jt, nck * 512 : (nck + 1) * 512],
                                start=(jt == 0),
                                stop=(jt == NT - 1),
                            )
                        nc.scalar.copy(
                            ztmp[:, nck * 512 : (nck + 1) * 512], psz[:]
                        )
                    znew = pzz.tile([128, NT, 64], BF, tag=f"z{r % 2}", name=f"znew{r}")
                    pzt = psum.tile([128, 1024], BF, tag="zt", bufs=1, name="pzt")
                    for jt in range(NT):
                        nc.tensor.transpose(
                            pzt[:, jt * 64 : (jt + 1) * 64],
                            ztmp[:, jt * 128 : (jt + 1) * 128],
                            ident_bf[0:64, 0:64],
                        )
                    for jt in range(NT):
                        nc.scalar.activation(
                            znew[:, jt, :],
                            pzt[:, jt * 64 : (jt + 1) * 64],
                            AF.Copy,
                            scale=rd_all[:, jt : jt + 1],
                        )
                    aslice = alpha_sb[:, r * HL + hl : r * HL + hl + 1]
                    rslice = res_row[:, :, hl * 64 : (hl + 1) * 64]
                    if r == 0:
                        nc.vector.tensor_scalar_mul(rslice, znew[:], aslice)
                    else:
                        nc.vector.scalar_tensor_tensor(
                            rslice, znew[:], aslice, rslice,
                            op0=ALU.mult, op1=ALU.add,
                        )
                    zprev = znew

        # ---- output projection (partial: this core's heads + 0.5*bout)
        with tc.tile_pool(name="po", bufs=2) as po_:
            resT = [po_.tile([128, N], FPR, tag=f"rT{f}", name=f"rT{f}") for f in range(2)]
            wout_r = []
            for f in range(2):
                t = po_.tile([128, 512], FPR, tag=f"wor{f}", name=f"wor{f}")
                nc.scalar.copy(t[:], wout_sb[f][:])
                wout_r.append(t)
            half_rr = po_.tile([1, 128], FPR)
            nc.scalar.copy(half_rr[:], half_r[:])
            bout_rr = po_.tile([1, 512], FPR)
            nc.scalar.copy(bout_rr[:], boutrow[:])
            for f in range(2):
                for g in range(4):
                    ps = psum.tile([128, 512], FP, tag="gen", name="psg")
                    for u in range(4):
                        jt = g * 4 + u
                        nc.tensor.transpose(
                            ps[:, u * 128 : (u + 1) * 128],
                            res_row[:, jt, f * 128 : (f + 1) * 128],
                            ident_f[:],
                        )
                    nc.scalar.copy(
                        resT[f][:, g * 512 : (g + 1) * 512], ps[:]
                    )
            for tt in range(NT):
                ps = psum.tile([128, 512], FP, tag="gen", name="psg")
                nc.tensor.matmul(
                    ps[:],
                    half_rr[:],
                    bout_rr[:],
                    start=True,
                    stop=False,
                )
                for f in range(2):
                    nc.tensor.matmul(
                        ps[:],
                        resT[f][:, tt * 128 : (tt + 1) * 128],
                        wout_r[f][:],
                        start=False,
                        stop=(f == 1),
                    )
                o_sb = po_.tile([128, 512], FP, tag="osb")
                nc.scalar.copy(o_sb[:], ps[:])
                nc.sync.dma_start(
                    out_d.ap()[tt * 128 : (tt + 1) * 128, :], o_sb[:]
                )

    nc.compile()
    return nc


_CACHE: dict = {}


def _in_maps(x, Wqkv, bqkv, Wout, bout, alphas_raw):
    maps = []
    for c in range(8):
        b, hg = c // 2, c % 2
        s, e = hg * 256, (hg + 1) * 256
        wqk = np.concatenate(
            [Wqkv[:, s:e], Wqkv[:, 512 + s : 512 + e]], axis=1
        )
        maps.append(
            {
                "xt": np.ascontiguousarray(x[b].T, np.float32),
                "wqk": np.ascontiguousarray(wqk, np.float32),
                "wv": np.ascontiguousarray(
                    Wqkv[:, 1024 + s : 1024 + e], np.float32
                ),
                "bqk": np.ascontiguousarray(
                    np.concatenate([bqkv[s:e], bqkv[512 + s : 512 + e]]),
                    np.float32,
                ),
                "bv": np.ascontiguousarray(
                    bqkv[None, 1024 + s : 1024 + e], np.float32
                ),
                "wout": np.ascontiguousarray(Wout[s:e, :], np.float32),
                "bout": np.ascontiguousarray(bout[None, :], np.float32),
                "araw": np.ascontiguousarray(
                    alphas_raw[:, hg * HL : (hg + 1) * HL].reshape(1, -1),
                    np.float32,
                ),
            }
        )
    return maps


def kernel(x, Wqkv, bqkv, Wout, bout, alphas_raw, _trace=False):
    x = np.asarray(x, np.float32)
    if "nc" not in _CACHE:
        _CACHE["nc"] = _build()
    nc = _CACHE["nc"]
    maps = _in_maps(
        np.asarray(x), np.asarray(Wqkv), np.asarray(bqkv),
        np.asarray(Wout), np.asarray(bout), np.asarray(alphas_raw),
    )
    kw = {}
    if _trace:
        kw = {"trace": True}
    res = run_bass_kernel_spmd(nc, maps, core_ids=list(range(8)), **kw)
    _CACHE["last_results"] = res
    out = np.empty((4, N, DIM), np.float32)
    for b in range(4):
        out[b] = res.results[2 * b]["out"] + res.results[2 * b + 1]["out"]
    return out


# revision 25
# speedup vs baseline: 1.0281x; 1.0281x over previous
"""AGF sparse attention (top-k=64 mask + softmax + 3-term polynomial filter)
on 8 TRN2 NeuronCores.

Sharding: core c -> batch b = c//2, head-group hg = c%2 (4 of 8 heads).
Each core runs the full per-(b, head-group) pipeline on-device:
  qkv projection -> scores (fp32r matmuls) -> exact-ish top-64 threshold via
  chunked max8 + match_replace -> masked softmax (unnormalized, fused
  mask*exp + row-sum) -> A^T via PE transposes -> 3x (A @ z) recurrence with
  per-token 1/rowsum normalization folded into the transpose-back copies ->
  partial output projection (+ 0.5*bout per core).
Host only shards inputs and sums the two partial outputs per batch element.
"""

import sys

sys.path.insert(0, "/opt/trn_rl_repo")

from contextlib import ExitStack  # noqa: E402

import numpy as np  # noqa: E402
import ml_dtypes  # noqa: E402

import concourse.bass as bass  # noqa: E402
import concourse.tile as tile  # noqa: E402
from concourse import bacc, mybir  # noqa: E402
from concourse.bass_utils import run_bass_kernel_spmd  # noqa: E402

FP = mybir.dt.float32
FPR = mybir.dt.float32r
BF = mybir.dt.bfloat16
AF = mybir.ActivationFunctionType
ALU = mybir.AluOpType

N, DIM = 2048, 512
H, HL, DH = 8, 4, 64  # total heads, local heads per core, head dim
NT = N // 128  # 16 token tiles
CC = DIM // 128  # 4 contraction chunks
ORDER = 3
NEG = -1.0e30
SCALE = DH**-0.5  # 0.125


def _build():
    nc = bacc.Bacc(
        "TRN2", target_bir_lowering=False, debug=False, num_devices=8
    )
    xt_d = nc.dram_tensor("xt", [DIM, N], FP, kind="ExternalInput")
    wqk_d = nc.dram_tensor("wqk", [DIM, 512], FP, kind="ExternalInput")
    wv_d = nc.dram_tensor("wv", [DIM, 256], FP, kind="ExternalInput")
    bqk_d = nc.dram_tensor("bqk", [512], FP, kind="ExternalInput")
    bv_d = nc.dram_tensor("bv", [1, 256], FP, kind="ExternalInput")
    wout_d = nc.dram_tensor("wout", [256, DIM], FP, kind="ExternalInput")
    bout_d = nc.dram_tensor("bout", [1, DIM], FP, kind="ExternalInput")
    ar_d = nc.dram_tensor("araw", [1, ORDER * HL], FP, kind="ExternalInput")
    out_d = nc.dram_tensor("out", [N, DIM], FP, kind="ExternalOutput")

    ident_bf_d = nc.inline_tensor(
        np.eye(128, dtype=ml_dtypes.bfloat16), name="identbf"
    )
    ident_f_d = nc.inline_tensor(np.eye(128, dtype=np.float32), name="identf")
    ones_d = nc.inline_tensor(np.ones((1, 128), np.float32), name="ones1")
    half_d = nc.inline_tensor(
        np.full((1, 128), 0.5, np.float32), name="half1"
    )

    with tile.TileContext(nc) as tc, ExitStack() as ctx:
        consts = ctx.enter_context(tc.tile_pool(name="consts", bufs=1))
        pw = ctx.enter_context(tc.tile_pool(name="weights", bufs=1))
        pqk = ctx.enter_context(tc.tile_pool(name="qkT", bufs=1))
        pv = ctx.enter_context(tc.tile_pool(name="vsb", bufs=1))
        pres = ctx.enter_context(tc.tile_pool(name="res", bufs=1))
        psum = ctx.enter_context(tc.tile_pool(name="psum", bufs=2, space="PSUM"))
        pS = ctx.enter_context(tc.tile_pool(name="pS", bufs=2))
        pA = ctx.enter_context(tc.tile_pool(name="pA", bufs=4))
        pzz = ctx.enter_context(tc.tile_pool(name="pzz", bufs=2))

        ident_bf = consts.tile([128, 128], BF)
        nc.sync.dma_start(ident_bf[:], ident_bf_d.ap())
        ident_f = consts.tile([128, 128], FP)
        nc.sync.dma_start(ident_f[:], ident_f_d.ap())
        ones_r = consts.tile([1, 128], FP)
        nc.sync.dma_start(ones_r[:], ones_d.ap())
        half_r = consts.tile([1, 128], FP)
        nc.sync.dma_start(half_r[:], half_d.ap())
        bvrow = consts.tile([1, 256], FP)
        nc.sync.dma_start(bvrow[:], bv_d.ap())
        boutrow = consts.tile([1, 512], FP)
        nc.sync.dma_start(boutrow[:], bout_d.ap())
        bqk_sb = consts.tile([128, 4], FP)
        nc.sync.dma_start(
            bqk_sb[:], bqk_d.ap().rearrange("(f p) -> p f", p=128)
        )
        araw_t = consts.tile([1, ORDER * HL], FP)
        nc.sync.dma_start(araw_t[:], ar_d.ap())
        alpha_g = consts.tile([1, ORDER * HL], FP)
        nc.scalar.activation(alpha_g[:], araw_t[:], AF.Gelu)
        alpha_sb = consts.tile([128, ORDER * HL], FP)
        nc.gpsimd.partition_broadcast(alpha_sb[:], alpha_g[:])

        wqk_sb = []
        wv_sb = []
        for c in range(CC):
            t = pw.tile([128, 512], FP, tag=f"wqk{c}", name=f"wqk{c}")
            nc.sync.dma_start(t[:], wqk_d.ap()[c * 128 : (c + 1) * 128, :])
            wqk_sb.append(t)
            t = pw.tile([128, 256], FP, tag=f"wv{c}", name=f"wv{c}")
            nc.sync.dma_start(t[:], wv_d.ap()[c * 128 : (c + 1) * 128, :])
            wv_sb.append(t)
        wout_sb = []
        for f in range(2):
            t = pw.tile([128, 512], FP, tag=f"wo{f}", name=f"wo{f}")
            nc.sync.dma_start(t[:], wout_d.ap()[f * 128 : (f + 1) * 128, :])
            wout_sb.append(t)

        # qkT tiles: ft 0..1 = q^T (heads 0-1, 2-3), ft 2..3 = k^T
        qkT = [pqk.tile([128, N], FP, tag=f"qkT{i}", name=f"qkT{i}") for i in range(4)]
        v_sb = pv.tile([128, NT, 256], BF)  # v rows, packed [t_lo, t_hi, f]
        res_row = pres.tile([128, NT, 256], FP)  # sum_r alpha_r z_r (rows)

        # ---- phase 1: load x, build x^T; phase 2: projections
        with tc.tile_pool(name="xload", bufs=1) as px:
            xT = [px.tile([128, N], FP, tag=f"xT{c}", name=f"xT{c}") for c in range(CC)]
            for c in range(CC):
                nc.sync.dma_start(
                    xT[c][:], xt_d.ap()[c * 128 : (c + 1) * 128, :]
                )

            for ft in (0, 2, 1, 3):
                for tq in range(4):
                    ps = psum.tile([128, 512], FP, tag="qk", bufs=1, name="psqk")
                    for c in range(CC):
                        nc.tensor.matmul(
                            ps[:],
                            wqk_sb[c][:, ft * 128 : (ft + 1) * 128],
                            xT[c][:, tq * 512 : (tq + 1) * 512],
                            start=(c == 0),
                            stop=(c == CC - 1),
                        )
                    nc.scalar.activation(
                        qkT[ft][:, tq * 512 : (tq + 1) * 512],
                        ps[:],
                        AF.Identity,
                        bias=bqk_sb[:, ft : ft + 1],
                    )
            for it in range(NT):
                psv = psum.tile([128, 512], FP, tag="gen", name="psg")
                ps = psv[:, 0:256]
                nc.tensor.matmul(
                    ps,
                    ones_r[:],
                    bvrow[:],
                    start=True,
                    stop=False,
                )
                for c in range(CC):
                    nc.tensor.matmul(
                        ps,
                        xT[c][:, it * 128 : (it + 1) * 128],
                        wv_sb[c][:],
                        start=False,
                        stop=(c == CC - 1),
                    )
                nc.scalar.copy(v_sb[:, it, :], ps)

        # ---- head loop
        with tc.tile_pool(name="pAT", bufs=1) as pAT:
            for hl in range(HL):
                qt_tile = qkT[hl // 2]
                kt_tile = qkT[2 + hl // 2]
                po = (hl % 2) * 64
                AT = pAT.tile([128, NT, N], BF, tag="AT")
                rd_all = pzz.tile([128, NT], FP, tag="rd")
                import contextlib
                for it in range(NT):
                    S_sb = pS.tile([128, N], FP, tag="S")
                    hoist = (
                        tc.high_priority() if hl == 0
                        else contextlib.nullcontext()
                    )
                    with hoist:
                        for jc in range(4):
                            ps = psum.tile([128, 512], FP, tag="S", name="psS")
                            nc.tensor.matmul(
                                ps[:],
                                qt_tile[
                                    po : po + 64, it * 128 : (it + 1) * 128
                                ],
                                kt_tile[
                                    po : po + 64, jc * 512 : (jc + 1) * 512
                                ],
                                start=True,
                                stop=True,
                            )
                            nc.scalar.copy(
                                S_sb[:, jc * 512 : (jc + 1) * 512], ps[:]
                            )
                    # top-64 threshold: chunk=64 top-8 candidates, then
                    # 8x (max8 + match_replace) on the 256 candidates
                    C = pS.tile([128, 256], FP, tag="C")
                    for ch in range(32):
                        nc.vector.max(
                            C[:, ch * 8 : (ch + 1) * 8],
                            S_sb[:, ch * 64 : (ch + 1) * 64],
                        )
                    mneg = pS.tile([128, 1], FP, tag="mn")
                    mrow = pS.tile([128, 1], FP, tag="mr")
                    C3 = C[:].rearrange("p (c e) -> p c e", e=8)
                    nc.vector.tensor_reduce(
                        mrow[:], C3[:, :, 0], axis=mybir.AxisListType.X,
                        op=ALU.max,
                    )
                    nc.vector.tensor_scalar_mul(mneg[:], mrow[:], -SCALE)
                    m8 = pS.tile([128, 8], FP, tag="m8")
                    for itr in range(8):
                        nc.vector.max(m8[:], C[:])
                        if itr < 7:
                            nc.vector.match_replace(C[:], m8[:], C[:], NEG)
                    tau = pS.tile([128, 1], FP, tag="tau")
                    nc.scalar.activation(
                        tau[:], m8[:, 7:8], AF.Exp,
                        bias=mneg[:, 0:1], scale=SCALE,
                    )
                    E_sb = pS.tile([128, N], FP, tag="E")
                    nc.scalar.activation(
                        E_sb[:], S_sb[:], AF.Exp,
                        bias=mneg[:, 0:1], scale=SCALE,
                    )
                    A_bf = pA.tile([128, N], BF, tag="A")
                    dsum = pS.tile([128, 1], FP, tag="d")
                    nc.vector.scalar_tensor_tensor(
                        A_bf[:],
                        E_sb[:],
                        tau[:, 0:1],
                        E_sb[:],
                        op0=ALU.is_ge,
                        op1=ALU.mult,
                        accum_out=dsum[:],
                    )
                    nc.vector.reciprocal(rd_all[:, it : it + 1], dsum[:])
                    nc.sync.dma_start_transpose(
                        AT[:, :, it * 128 : (it + 1) * 128], A_bf[:]
                    )
                # recurrence: z_r = rd * (A_u @ z_{r-1}), res += alpha_r z_r
                zprev = None
                for r in range(ORDER):
                    ztmp = pzz.tile([64, N], BF, tag="ztmp")
                    for nck in range(4):
                        psz = psum.tile([64, 512], FP, tag="z", name="psz")
                        for jt in range(NT):
                            lhsT = (
                                v_sb[:, jt, hl * 64 : (hl + 1) * 64]
                                if r == 0
                                else zprev[:, jt, :]
                            )
                            nc.tensor.matmul(
                                psz[:],
                                lhsT,
                                AT[:, jt, nck * 512 : (nck + 1) * 512],
                                start=(jt == 0),
                                stop=(jt == NT - 1),
                            )
                        nc.scalar.copy(
                            ztmp[:, nck * 512 : (nck + 1) * 512], psz[:]
                        )
                    znew = pzz.tile([128, NT, 64], BF, tag=f"z{r % 2}", name=f"znew{r}")
                    pzt = psum.tile([128, 1024], BF, tag="zt", bufs=1, name="pzt")
                    for jt in range(NT):
                        nc.tensor.transpose(
                            pzt[:, jt * 64 : (jt + 1) * 64],
                            ztmp[:, jt * 128 : (jt + 1) * 128],
                            ident_bf[0:64, 0:64],
                        )
                    for jt in range(NT):
                        nc.scalar.activation(
                            znew[:, jt, :],
                            pzt[:, jt * 64 : (jt + 1) * 64],
                            AF.Copy,
                            scale=rd_all[:, jt : jt + 1],
                        )
                    aslice = alpha_sb[:, r * HL + hl : r * HL + hl + 1]
                    rslice = res_row[:, :, hl * 64 : (hl + 1) * 64]
                    if r == 0:
                        nc.vector.tensor_scalar_mul(rslice, znew[:], aslice)
                    else:
                        nc.vector.scalar_tensor_tensor(
                            rslice, znew[:], aslice, rslice,
                            op0=ALU.mult, op1=ALU.add,
                        )
                    zprev = znew

        # ---- output projection (partial: this core's heads + 0.5*bout)
        with tc.tile_pool(name="po", bufs=2) as po_:
            resT = [po_.tile([128, N], FPR, tag=f"rT{f}", name=f"rT{f}") for f in range(2)]
            wout_r = []
            for f in range(2):
                t = po_.tile([128, 512], FPR, tag=f"wor{f}", name=f"wor{f}")
                nc.scalar.copy(t[:], wout_sb[f][:])
                wout_r.append(t)
            half_rr = po_.tile([1, 128], FPR)
            nc.scalar.copy(half_rr[:], half_r[:])
            bout_rr = po_.tile([1, 512], FPR)
            nc.scalar.copy(bout_rr[:], boutrow[:])
            for f in range(2):
                for g in range(4):
                    ps = psum.tile([128, 512], FP, tag="gen", name="psg")
                    for u in range(4):
                        jt = g * 4 + u
                        nc.tensor.transpose(
                            ps[:, u * 128 : (u + 1) * 128],
                            res_row[:, jt, f * 128 : (f + 1) * 128],
                            ident_f[:],
                        )
                    nc.scalar.copy(
                        resT[f][:, g * 512 : (g + 1) * 512], ps[:]
                    )
            for tt in range(NT):
                ps = psum.tile([128, 512], FP, tag="gen", name="psg")
                nc.tensor.matmul(
                    ps[:],
                    half_rr[:],
                    bout_rr[:],
                    start=True,
                    stop=False,
                )
                for f in range(2):
                    nc.tensor.matmul(
                        ps[:],
                        resT[f][:, tt * 128 : (tt + 1) * 128],
                        wout_r[f][:],
                        start=False,
                        stop=(f == 1),
                    )
                o_sb = po_.tile([128, 512], FP, tag="osb")
                nc.scalar.copy(o_sb[:], ps[:])
                nc.sync.dma_start(
                    out_d.ap()[tt * 128 : (tt + 1) * 128, :], o_sb[:]
                )

    nc.compile()
    return nc


_CACHE: dict = {}


def _in_maps(x, Wqkv, bqkv, Wout, bout, alphas_raw):
    maps = []
    for c in range(8):
        b, hg = c // 2, c % 2
        s, e = hg * 256, (hg + 1) * 256
        wqk = np.concatenate(
            [Wqkv[:, s:e], Wqkv[:, 512 + s : 512 + e]], axis=1
        )
        maps.append(
            {
                "xt": np.ascontiguousarray(x[b].T, np.float32),
                "wqk": np.ascontiguousarray(wqk, np.float32),
                "wv": np.ascontiguousarray(
                    Wqkv[:, 1024 + s : 1024 + e], np.float32
                ),
                "bqk": np.ascontiguousarray(
                    np.concatenate([bqkv[s:e], bqkv[512 + s : 512 + e]]),
                    np.float32,
                ),
                "bv": np.ascontiguousarray(
                    bqkv[None, 1024 + s : 1024 + e], np.float32
                ),
                "wout": np.ascontiguousarray(Wout[s:e, :], np.float32),
                "bout": np.ascontiguousarray(bout[None, :], np.float32),
                "araw": np.ascontiguousarray(
                    alphas_raw[:, hg * HL : (hg + 1) * HL].reshape(1, -1),
                    np.float32,
                ),
            }
        )
    return maps


def kernel(x, Wqkv, bqkv, Wout, bout, alphas_raw, _trace=False):
    x = np.asarray(x, np.float32)
    if "nc" not in _CACHE:
        _CACHE["nc"] = _build()
    nc = _CACHE["nc"]
    maps = _in_maps(
        np.asarray(x), np.asarray(Wqkv), np.asarray(bqkv),
        np.asarray(Wout), np.asarray(bout), np.asarray(alphas_raw),
    )
    kw = {}
    if _trace:
        kw = {"trace": True}
    res = run_bass_kernel_spmd(nc, maps, core_ids=list(range(8)), **kw)
    _CACHE["last_results"] = res
    out = np.empty((4, N, DIM), np.float32)
    for b in range(4):
        out[b] = res.results[2 * b]["out"] + res.results[2 * b + 1]["out"]
    return out


# revision 26
# speedup vs baseline: 1.0438x; 1.0152x over previous
"""AGF sparse attention (top-k=64 mask + softmax + 3-term polynomial filter)
on 8 TRN2 NeuronCores.

Sharding: core c -> batch b = c//2, head-group hg = c%2 (4 of 8 heads).
Each core runs the full per-(b, head-group) pipeline on-device:
  qkv projection -> scores (fp32r matmuls) -> exact-ish top-64 threshold via
  chunked max8 + match_replace -> masked softmax (unnormalized, fused
  mask*exp + row-sum) -> A^T via PE transposes -> 3x (A @ z) recurrence with
  per-token 1/rowsum normalization folded into the transpose-back copies ->
  partial output projection (+ 0.5*bout per core).
Host only shards inputs and sums the two partial outputs per batch element.
"""

import sys

sys.path.insert(0, "/opt/trn_rl_repo")

from contextlib import ExitStack  # noqa: E402

import numpy as np  # noqa: E402
import ml_dtypes  # noqa: E402

import concourse.bass as bass  # noqa: E402
import concourse.tile as tile  # noqa: E402
from concourse import bacc, mybir  # noqa: E402
from concourse.bass_utils import run_bass_kernel_spmd  # noqa: E402

FP = mybir.dt.float32
FPR = mybir.dt.float32r
BF = mybir.dt.bfloat16
AF = mybir.ActivationFunctionType
ALU = mybir.AluOpType

N, DIM = 2048, 512
H, HL, DH = 8, 4, 64  # total heads, local heads per core, head dim
NT = N // 128  # 16 token tiles
CC = DIM // 128  # 4 contraction chunks
ORDER = 3
NEG = -1.0e30
SCALE = DH**-0.5  # 0.125


def _build():
    nc = bacc.Bacc(
        "TRN2", target_bir_lowering=False, debug=False, num_devices=8
    )
    xt_d = nc.dram_tensor("xt", [DIM, N], FP, kind="ExternalInput")
    wqk_d = nc.dram_tensor("wqk", [DIM, 512], FP, kind="ExternalInput")
    wv_d = nc.dram_tensor("wv", [DIM, 256], FP, kind="ExternalInput")
    bqk_d = nc.dram_tensor("bqk", [512], FP, kind="ExternalInput")
    bv_d = nc.dram_tensor("bv", [1, 256], FP, kind="ExternalInput")
    wout_d = nc.dram_tensor("wout", [256, DIM], FP, kind="ExternalInput")
    bout_d = nc.dram_tensor("bout", [1, DIM], FP, kind="ExternalInput")
    ar_d = nc.dram_tensor("araw", [1, ORDER * HL], FP, kind="ExternalInput")
    out_d = nc.dram_tensor("out", [N, DIM], FP, kind="ExternalOutput")

    ident_bf_d = nc.inline_tensor(
        np.eye(128, dtype=ml_dtypes.bfloat16), name="identbf"
    )
    ident_f_d = nc.inline_tensor(np.eye(128, dtype=np.float32), name="identf")
    ones_d = nc.inline_tensor(np.ones((1, 128), np.float32), name="ones1")
    half_d = nc.inline_tensor(
        np.full((1, 128), 0.5, np.float32), name="half1"
    )

    with tile.TileContext(nc) as tc, ExitStack() as ctx:
        consts = ctx.enter_context(tc.tile_pool(name="consts", bufs=1))
        pw = ctx.enter_context(tc.tile_pool(name="weights", bufs=1))
        pqk = ctx.enter_context(tc.tile_pool(name="qkT", bufs=1))
        pv = ctx.enter_context(tc.tile_pool(name="vsb", bufs=1))
        pres = ctx.enter_context(tc.tile_pool(name="res", bufs=1))
        psum = ctx.enter_context(tc.tile_pool(name="psum", bufs=2, space="PSUM"))
        pAD = ctx.enter_context(tc.tile_pool(name="adram", bufs=2, space="DRAM"))
        pS = ctx.enter_context(tc.tile_pool(name="pS", bufs=2))
        pA = ctx.enter_context(tc.tile_pool(name="pA", bufs=3))
        pzz = ctx.enter_context(tc.tile_pool(name="pzz", bufs=2))

        ident_bf = consts.tile([128, 128], BF)
        nc.sync.dma_start(ident_bf[:], ident_bf_d.ap())
        ident_f = consts.tile([128, 128], FP)
        nc.sync.dma_start(ident_f[:], ident_f_d.ap())
        ones_r = consts.tile([1, 128], FP)
        nc.sync.dma_start(ones_r[:], ones_d.ap())
        half_r = consts.tile([1, 128], FP)
        nc.sync.dma_start(half_r[:], half_d.ap())
        bvrow = consts.tile([1, 256], FP)
        nc.sync.dma_start(bvrow[:], bv_d.ap())
        boutrow = consts.tile([1, 512], FP)
        nc.sync.dma_start(boutrow[:], bout_d.ap())
        bqk_sb = consts.tile([128, 4], FP)
        nc.sync.dma_start(
            bqk_sb[:], bqk_d.ap().rearrange("(f p) -> p f", p=128)
        )
        araw_t = consts.tile([1, ORDER * HL], FP)
        nc.sync.dma_start(araw_t[:], ar_d.ap())
        alpha_g = consts.tile([1, ORDER * HL], FP)
        nc.scalar.activation(alpha_g[:], araw_t[:], AF.Gelu)
        alpha_sb = consts.tile([128, ORDER * HL], FP)
        nc.gpsimd.partition_broadcast(alpha_sb[:], alpha_g[:])

        wqk_sb = []
        wv_sb = []
        for c in range(CC):
            t = pw.tile([128, 512], FP, tag=f"wqk{c}", name=f"wqk{c}")
            nc.sync.dma_start(t[:], wqk_d.ap()[c * 128 : (c + 1) * 128, :])
            wqk_sb.append(t)
            t = pw.tile([128, 256], FP, tag=f"wv{c}", name=f"wv{c}")
            nc.sync.dma_start(t[:], wv_d.ap()[c * 128 : (c + 1) * 128, :])
            wv_sb.append(t)
        wout_sb = []
        for f in range(2):
            t = pw.tile([128, 512], FP, tag=f"wo{f}", name=f"wo{f}")
            nc.sync.dma_start(t[:], wout_d.ap()[f * 128 : (f + 1) * 128, :])
            wout_sb.append(t)

        # qkT tiles: ft 0..1 = q^T (heads 0-1, 2-3), ft 2..3 = k^T
        qkT = [pqk.tile([128, N], FP, tag=f"qkT{i}", name=f"qkT{i}") for i in range(4)]
        v_sb = pv.tile([128, NT, 256], BF)  # v rows, packed [t_lo, t_hi, f]
        res_row = pres.tile([128, NT, 256], FP)  # sum_r alpha_r z_r (rows)

        # ---- phase 1: load x, build x^T; phase 2: projections
        with tc.tile_pool(name="xload", bufs=1) as px:
            xT = [px.tile([128, N], FP, tag=f"xT{c}", name=f"xT{c}") for c in range(CC)]
            for c in range(CC):
                nc.sync.dma_start(
                    xT[c][:], xt_d.ap()[c * 128 : (c + 1) * 128, :]
                )

            for ft in (0, 2, 1, 3):
                for tq in range(4):
                    ps = psum.tile([128, 512], FP, tag="qk", bufs=1, name="psqk")
                    for c in range(CC):
                        nc.tensor.matmul(
                            ps[:],
                            wqk_sb[c][:, ft * 128 : (ft + 1) * 128],
                            xT[c][:, tq * 512 : (tq + 1) * 512],
                            start=(c == 0),
                            stop=(c == CC - 1),
                        )
                    nc.scalar.activation(
                        qkT[ft][:, tq * 512 : (tq + 1) * 512],
                        ps[:],
                        AF.Identity,
                        bias=bqk_sb[:, ft : ft + 1],
                    )
            for it in range(NT):
                psv = psum.tile([128, 512], FP, tag="gen", name="psg")
                ps = psv[:, 0:256]
                nc.tensor.matmul(
                    ps,
                    ones_r[:],
                    bvrow[:],
                    start=True,
                    stop=False,
                )
                for c in range(CC):
                    nc.tensor.matmul(
                        ps,
                        xT[c][:, it * 128 : (it + 1) * 128],
                        wv_sb[c][:],
                        start=False,
                        stop=(c == CC - 1),
                    )
                nc.scalar.copy(v_sb[:, it, :], ps)

        # ---- head loop
        with tc.tile_pool(name="pAT", bufs=1) as pAT:
            for hl in range(HL):
                qt_tile = qkT[hl // 2]
                kt_tile = qkT[2 + hl // 2]
                po = (hl % 2) * 64
                AT = pAT.tile([128, NT, N], BF, tag="AT")
                A_dram = pAD.tile([N, N], BF, tag="ad", name="adram")
                rd_all = pzz.tile([128, NT], FP, tag="rd")
                import contextlib
                for it in range(NT):
                    S_sb = pS.tile([128, N], FP, tag="S")
                    hoist = (
                        tc.high_priority() if hl == 0
                        else contextlib.nullcontext()
                    )
                    with hoist:
                        for jc in range(4):
                            ps = psum.tile([128, 512], FP, tag="S", name="psS")
                            nc.tensor.matmul(
                                ps[:],
                                qt_tile[
                                    po : po + 64, it * 128 : (it + 1) * 128
                                ],
                                kt_tile[
                                    po : po + 64, jc * 512 : (jc + 1) * 512
                                ],
                                start=True,
                                stop=True,
                            )
                            nc.scalar.copy(
                                S_sb[:, jc * 512 : (jc + 1) * 512], ps[:]
                            )
                    # top-64 threshold: chunk=64 top-8 candidates, then
                    # 8x (max8 + match_replace) on the 256 candidates
                    C = pS.tile([128, 256], FP, tag="C", bufs=5)
                    for ch in range(32):
                        nc.vector.max(
                            C[:, ch * 8 : (ch + 1) * 8],
                            S_sb[:, ch * 64 : (ch + 1) * 64],
                        )
                    mneg = pS.tile([128, 1], FP, tag="mn", bufs=5)
                    mrow = pS.tile([128, 1], FP, tag="mr", bufs=5)
                    C3 = C[:].rearrange("p (c e) -> p c e", e=8)
                    nc.vector.tensor_reduce(
                        mrow[:], C3[:, :, 0], axis=mybir.AxisListType.X,
                        op=ALU.max,
                    )
                    nc.vector.tensor_scalar_mul(mneg[:], mrow[:], -SCALE)
                    m8 = pS.tile([128, 8], FP, tag="m8", bufs=5)
                    for itr in range(8):
                        nc.vector.max(m8[:], C[:])
                        if itr < 7:
                            nc.vector.match_replace(C[:], m8[:], C[:], NEG)
                    tau = pS.tile([128, 1], FP, tag="tau", bufs=5)
                    nc.scalar.activation(
                        tau[:], m8[:, 7:8], AF.Exp,
                        bias=mneg[:, 0:1], scale=SCALE,
                    )
                    E_sb = pS.tile([128, N], FP, tag="E")
                    nc.scalar.activation(
                        E_sb[:], S_sb[:], AF.Exp,
                        bias=mneg[:, 0:1], scale=SCALE,
                    )
                    A_bf = pA.tile([128, N], BF, tag="A")
                    dsum = pS.tile([128, 1], FP, tag="d", bufs=5)
                    nc.vector.scalar_tensor_tensor(
                        A_bf[:],
                        E_sb[:],
                        tau[:, 0:1],
                        E_sb[:],
                        op0=ALU.is_ge,
                        op1=ALU.mult,
                        accum_out=dsum[:],
                    )
                    nc.vector.reciprocal(rd_all[:, it : it + 1], dsum[:])
                    nc.sync.dma_start(
                        A_dram[it * 128 : (it + 1) * 128, :], A_bf[:]
                    )
                    nc.sync.dma_start_transpose(
                        AT[:, :, it * 128 : (it + 1) * 128],
                        A_dram[it * 128 : (it + 1) * 128, :],
                    )
                # recurrence: z_r = rd * (A_u @ z_{r-1}), res += alpha_r z_r
                zprev = None
                for r in range(ORDER):
                    ztmp = pzz.tile([64, N], BF, tag="ztmp")
                    for nck in range(4):
                        psz = psum.tile([64, 512], FP, tag="z", name="psz")
                        for jt in range(NT):
                            lhsT = (
                                v_sb[:, jt, hl * 64 : (hl + 1) * 64]
                                if r == 0
                                else zprev[:, jt, :]
                            )
                            nc.tensor.matmul(
                                psz[:],
                                lhsT,
                                AT[:, jt, nck * 512 : (nck + 1) * 512],
                                start=(jt == 0),
                                stop=(jt == NT - 1),
                            )
                        nc.scalar.copy(
                            ztmp[:, nck * 512 : (nck + 1) * 512], psz[:]
                        )
                    znew = pzz.tile([128, NT, 64], BF, tag=f"z{r % 2}", name=f"znew{r}")
                    pzt = psum.tile([128, 1024], BF, tag="zt", bufs=1, name="pzt")
                    for jt in range(NT):
                        nc.tensor.transpose(
                            pzt[:, jt * 64 : (jt + 1) * 64],
                            ztmp[:, jt * 128 : (jt + 1) * 128],
                            ident_bf[0:64, 0:64],
                        )
                    for jt in range(NT):
                        nc.scalar.activation(
                            znew[:, jt, :],
                            pzt[:, jt * 64 : (jt + 1) * 64],
                            AF.Copy,
                            scale=rd_all[:, jt : jt + 1],
                        )
                    aslice = alpha_sb[:, r * HL + hl : r * HL + hl + 1]
                    rslice = res_row[:, :, hl * 64 : (hl + 1) * 64]
                    if r == 0:
                        nc.vector.tensor_scalar_mul(rslice, znew[:], aslice)
                    else:
                        nc.vector.scalar_tensor_tensor(
                            rslice, znew[:], aslice, rslice,
                            op0=ALU.mult, op1=ALU.add,
                        )
                    zprev = znew

        # ---- output projection (partial: this core's heads + 0.5*bout)
        with tc.tile_pool(name="po", bufs=2) as po_:
            resT = [po_.tile([128, N], FPR, tag=f"rT{f}", name=f"rT{f}") for f in range(2)]
            wout_r = []
            for f in range(2):
                t = po_.tile([128, 512], FPR, tag=f"wor{f}", name=f"wor{f}")
                nc.scalar.copy(t[:], wout_sb[f][:])
                wout_r.append(t)
            half_rr = po_.tile([1, 128], FPR)
            nc.scalar.copy(half_rr[:], half_r[:])
            bout_rr = po_.tile([1, 512], FPR)
            nc.scalar.copy(bout_rr[:], boutrow[:])
            for f in range(2):
                for g in range(4):
                    ps = psum.tile([128, 512], FP, tag="gen", name="psg")
                    for u in range(4):
                        jt = g * 4 + u
                        nc.tensor.transpose(
                            ps[:, u * 128 : (u + 1) * 128],
                            res_row[:, jt, f * 128 : (f + 1) * 128],
                            ident_f[:],
                        )
                    nc.scalar.copy(
                        resT[f][:, g * 512 : (g + 1) * 512], ps[:]
                    )
            for tt in range(NT):
                ps = psum.tile([128, 512], FP, tag="gen", name="psg")
                nc.tensor.matmul(
                    ps[:],
                    half_rr[:],
                    bout_rr[:],
                    start=True,
                    stop=False,
                )
                for f in range(2):
                    nc.tensor.matmul(
                        ps[:],
                        resT[f][:, tt * 128 : (tt + 1) * 128],
                        wout_r[f][:],
                        start=False,
                        stop=(f == 1),
                    )
                o_sb = po_.tile([128, 512], FP, tag="osb")
                nc.scalar.copy(o_sb[:], ps[:])
                nc.sync.dma_start(
                    out_d.ap()[tt * 128 : (tt + 1) * 128, :], o_sb[:]
                )

    nc.compile()
    return nc


_CACHE: dict = {}


def _in_maps(x, Wqkv, bqkv, Wout, bout, alphas_raw):
    maps = []
    for c in range(8):
        b, hg = c // 2, c % 2
        s, e = hg * 256, (hg + 1) * 256
        wqk = np.concatenate(
            [Wqkv[:, s:e], Wqkv[:, 512 + s : 512 + e]], axis=1
        )
        maps.append(
            {
                "xt": np.ascontiguousarray(x[b].T, np.float32),
                "wqk": np.ascontiguousarray(wqk, np.float32),
                "wv": np.ascontiguousarray(
                    Wqkv[:, 1024 + s : 1024 + e], np.float32
                ),
                "bqk": np.ascontiguousarray(
                    np.concatenate([bqkv[s:e], bqkv[512 + s : 512 + e]]),
                    np.float32,
                ),
                "bv": np.ascontiguousarray(
                    bqkv[None, 1024 + s : 1024 + e], np.float32
                ),
                "wout": np.ascontiguousarray(Wout[s:e, :], np.float32),
                "bout": np.ascontiguousarray(bout[None, :], np.float32),
                "araw": np.ascontiguousarray(
                    alphas_raw[:, hg * HL : (hg + 1) * HL].reshape(1, -1),
                    np.float32,
                ),
            }
        )
    return maps


def kernel(x, Wqkv, bqkv, Wout, bout, alphas_raw, _trace=False):
    x = np.asarray(x, np.float32)
    if "nc" not in _CACHE:
        _CACHE["nc"] = _build()
    nc = _CACHE["nc"]
    maps = _in_maps(
        np.asarray(x), np.asarray(Wqkv), np.asarray(bqkv),
        np.asarray(Wout), np.asarray(bout), np.asarray(alphas_raw),
    )
    kw = {}
    if _trace:
        kw = {"trace": True}
    res = run_bass_kernel_spmd(nc, maps, core_ids=list(range(8)), **kw)
    _CACHE["last_results"] = res
    out = np.empty((4, N, DIM), np.float32)
    for b in range(4):
        out[b] = res.results[2 * b]["out"] + res.results[2 * b + 1]["out"]
    return out
